# revision 27
# baseline (speedup 1.0000x reference)
"""BitLinear (fake-quant straight-through) Trainium2 kernel.

Math (per the reference nn module):
  dqx = round(x * s_x) / s_x         s_x = 127 / clip(rowabsmax(x), 1e-5)   (per token row)
  dqw = clip(round(w * s_w), -1, 1) / s_w    s_w = 1 / clip(mean(|w|), 1e-5)  (per tensor)
  out = dqx @ dqw.T + bias

Key facts this kernel exploits:
  * round(x*s_x) is an integer in [-127, 127] and clip(round(w*s_w)) is in
    {-1, 0, 1}; both are EXACT in bf16, and the matmul accumulates in fp32
    PSUM where all partial sums are exact integers.  The heavy matmul runs at
    bf16 PE rate; the per-token scale is applied to the exact integer matmul
    result at PSUM-evacuation time.
  * round-half-even == fp32 RNE, so `round(v)` is computed exactly as
    `(v + 1.5*2^23) - 1.5*2^23` with two fp32 ALU stages.
  * The tolerance gate is 2e-2; measured end-to-end error of this kernel is
    ~8.6e-3, dominated by two deliberate approximations: x is shipped to the
    device as bf16 (halves the input DMA; perturbs ~6% of the round-to-int
    decisions by +-1 step), and the output is stored as bf16 (+-2^-9
    relative) then upcast on the host.

Sharding: data parallel over the batch dim; core i computes batch element i
with the full weight.  No collectives; the host scatters x and gathers out.

Host-side static prep (weights are per-call constants): s_w (computed with
the same fp32 jnp.mean as the reference -- a 1e-6 error in s_w flips ternary
weights), the ternary weight pre-transposed into the matmul rhs layout
(bf16, exact), the bias broadcast row, and k1 = 1/(127*s_w).

Performance structure (~172-180 us/core; 512 matmuls x 216 ns = 110.6 us PE
floor, plus ~16 us fixed framework preamble/teardown):
  * Tokens are processed in quads (4 x 128 = 512 tokens, 8 quads/core); the
    coarse granularity keeps the DMA count at ~3 per 13.8 us PE slot, which
    keeps the tile framework's recycled DMA-completion semaphores far apart
    (fine-grained variants serialized on those recycle barriers).
  * The PE_HAM clock gate runs the PE at 1.2 GHz until it has been busy
    ~3.4 us, and re-throttles after any ~3.4 us idle gap -- so the matmul
    stream must never gap, and 40 dummy matmuls on scratch data pre-warm the
    gate through the input lead-in.
  * The input stage for quad q+3 is emitted alongside the matmuls for quad q;
    the first two quads are processed in per-128-token slices (with the
    -MAGIC convert on DVE) so the PE can start ~14 us in; steady-state quads
    use batched ops (convert on the scalar engine).
  * Engine queue steady-state budgets per 13.8 us quad slot: DVE ~9 us
    (absmax reduce, scales, round, 2 bias adds), ACT ~9.3 us (4 converts,
    4 wide PSUM evacuations), Pool ~8 us (2 bias adds, 4 output stores),
    sync ~8 us (x load + quad transpose dispatch).  PSUM uses all 8 banks
    (4 x [128,1024] tiles).
  * DMA rings: sync carries x loads + qx transposes, gpsimd carries the
    one-time weight load + output stores, scalar carries the lead-in x
    slices (so the DMA-semaphore ring never chains a transpose behind a
    steady-state x load).
"""

import numpy as np

from concourse import bacc, bass, mybir, tile
from concourse.bass_utils import run_bass_kernel_spmd

F32 = mybir.dt.float32
BF16 = mybir.dt.bfloat16
ALU = mybir.AluOpType
ACTF = mybir.ActivationFunctionType

MAGIC = 12582912.0  # 1.5 * 2**23: fp32 RNE round-to-integer constant
EPS = 1e-05

B, S, K, N = 8, 4096, 1024, 1024
N_CORES = 8
QS = 4       # token tiles per quad
PIPE = 3     # input-stage lookahead (quads)
WARMUP = 48  # dummy matmuls: pre-warm the PE HAM clock gate through
             # the lead-in (the first two quads are processed per-slice)


def build(s_tokens=S, k=K, n=N):
    """Build the single-core SPMD program: x[s_tokens,k] @ w[n,k]^T quantized."""
    nc = bacc.Bacc("TRN2", target_bir_lowering=False, debug=False)

    KT = k // 128          # contraction tiles
    NT = n // 128          # weight row tiles
    NH = n // 512          # psum-bank halves of the output feature dim
    NQ = s_tokens // (128 * QS)  # quads

    x_d = nc.dram_tensor("x", [s_tokens, k], BF16, kind="ExternalInput").ap()
    # qwt: host-ternarized weight, pre-transposed to the rhs layout
    # [kpart, kt, nt, n128]: element (p, kt, nt, j) = qw[n=nt*128+j, k=kt*128+p]
    qwt_d = nc.dram_tensor("qwt", [128, KT * n], BF16, kind="ExternalInput").ap()
    # bias broadcast to all 128 partitions (bf16)
    biasb_d = nc.dram_tensor("biasb", [128, n], BF16, kind="ExternalInput").ap()
    consts_d = nc.dram_tensor("consts", [128, 2], F32, kind="ExternalInput").ap()
    out_d = nc.dram_tensor("out", [s_tokens, n], BF16, kind="ExternalOutput").ap()

    x_q = x_d.rearrange("(q s p) k -> q p s k", s=QS, p=128)
    out_q = out_d.rearrange("(q s p) n -> q p s n", s=QS, p=128)

    with tile.TileContext(nc) as tc:
        with (
            tc.tile_pool(name="static", bufs=1) as static,
            tc.tile_pool(name="xpool", bufs=4) as xpool,
            tc.tile_pool(name="qpool", bufs=3) as qpool,
            tc.tile_pool(name="qtpool", bufs=4) as qtpool,
            tc.tile_pool(name="opool", bufs=4) as opool,
            tc.tile_pool(name="vpool", bufs=5) as vpool,
            tc.tile_pool(name="psum", bufs=4, space="PSUM") as psum_pool,
        ):
            # PE warmup scratch: memset by gpsimd, consumed by dummy matmuls
            scratch = static.tile([128, 512], BF16)
            nc.gpsimd.memset(scratch[:], 0.0)

            consts = static.tile([128, 2], F32)
            nc.scalar.dma_start(consts[:], consts_d[:])
            k1 = consts[:, 0:1]  # (1/s_w) / 127  (output scale factor)

            # weight rhs [128, kt, nt, 128] + bias on the gpsimd ring
            # (one-time; keeps the sync ring free for x loads + transposes)
            qwt = static.tile([128, KT, NT, 128], BF16)
            qwt_f = qwt[:].rearrange("p kt nt j -> p (kt nt j)")
            CH = KT * n // 2
            nc.gpsimd.dma_start(qwt_f[:, 0:CH], qwt_d[:, 0:CH])
            biasb = static.tile([128, n], BF16)
            nc.gpsimd.dma_start(biasb[:], biasb_d[:])
            nc.gpsimd.dma_start(qwt_f[:, CH:], qwt_d[:, CH:])

            # dummy matmuls: keep the PE busy through the input lead-in so
            # the HAM clock gate is warm (2.4 GHz) when the real stream
            # starts.  K=1 operands: PE-busy time is set by the 512 output
            # columns (216 ns each) but the moving operand is a single
            # partition row (~1 KB), so the dummies do not steal SBUF
            # bandwidth from the concurrent input-prep engines.
            ps_warm = psum_pool.tile([128, n], F32, name="ps_warm", tag="ps")
            for _ in range(WARMUP):
                nc.tensor.matmul(
                    ps_warm[:, 0:512], scratch[0:1, 0:128], scratch[0:1, :],
                    start=True, stop=True,
                )

            xs, fss, qxTs = {}, {}, {}

            def input_stage(q, split):
                """Load + quantize + transpose quad q.

                split=True processes the quad in 128-token slices (shorter
                dependency chains -- used for the lead-in quads); False uses
                one batched op per stage (fewer instructions, steady state).
                """
                x_s = xs[q] = xpool.tile([128, QS, k], BF16, name="x_s")
                cc = vpool.tile([128, QS], F32, name="cc")
                rc = vpool.tile([128, QS], F32, name="rc")
                ss = vpool.tile([128, QS], F32, name="ss")
                fs = fss[q] = vpool.tile([128, QS], F32, name="fs")
                xr = qpool.tile([128, QS, k], F32, name="xr")
                qx = qpool.tile([128, QS, k], BF16, name="qx")
                qxT = qxTs[q] = qtpool.tile([128, QS, KT, 128], BF16, name="qxT")

                ranges = [(s, s + 1) for s in range(QS)] if split else [(0, QS)]
                xeng = nc.scalar if split else nc.sync
                for lo, hi in ranges:
                    xeng.dma_start(x_s[:, lo:hi], x_q[q][:, lo:hi])
                    c = vpool.tile([128, hi - lo], F32, name=f"c{lo}" if split else "c")
                    nc.vector.tensor_reduce(
                        c[:], x_s[:, lo:hi], mybir.AxisListType.X, ALU.max,
                        apply_absolute_value=True,
                    )
                    nc.vector.tensor_scalar_max(cc[:, lo:hi], c[:], EPS)
                    nc.vector.reciprocal(rc[:, lo:hi], cc[:, lo:hi])
                    nc.vector.tensor_scalar_mul(ss[:, lo:hi], rc[:, lo:hi], 127.0)
                    nc.vector.tensor_scalar_mul(fs[:, lo:hi], cc[:, lo:hi], k1)
                    # round(x*s_x) via magic constant into an f32 scratch
                    # (per 128-token slice: the scalar operand is per-partition)
                    for s in range(lo, hi):
                        nc.vector.tensor_scalar(
                            xr[:, s, :], x_s[:, s, :], ss[:, s:s + 1], MAGIC,
                            ALU.mult, ALU.add,
                        )
                    for s in range(lo, hi):
                        if split:
                            nc.vector.tensor_scalar_sub(qx[:, s], xr[:, s], MAGIC)
                        else:
                            nc.scalar.activation(
                                qx[:, s], xr[:, s], ACTF.Copy, bias=-MAGIC,
                            )
                    # xbar transpose: [128s, k] -> [128k, KT, 128s] per slice
                    nc.sync.dma_start_transpose(qxT[:, lo:hi], qx[:, lo:hi])

            def mm_stage(q):
                qxT = qxTs.pop(q)
                fs = fss.pop(q)
                outs = opool.tile([128, QS, n], BF16, name="outs")
                for s in range(QS):
                    ps = psum_pool.tile([128, n], F32, name="ps", tag="ps")
                    for kt in range(KT):
                        for h in range(NH):
                            nc.tensor.matmul(
                                ps[:, h * 512:(h + 1) * 512],
                                qxT[:, s, kt, :],
                                qwt[:, kt, 4 * h:4 * h + 4, :],
                                start=(kt == 0),
                                stop=(kt == KT - 1),
                            )
                    nc.scalar.activation(
                        outs[:, s, :], ps[:], ACTF.Copy, scale=fs[:, s:s + 1],
                    )
                    beng = nc.gpsimd if s % 2 == 0 else nc.vector
                    beng.tensor_add(outs[:, s, :], outs[:, s, :], biasb[:])
                    nc.gpsimd.dma_start(out_q[q][:, s], outs[:, s])

            def fillers(nf):
                # K=1 dummies (see warmup): bridge early input-side deficits
                # without stealing SBUF bandwidth from the input prep
                psf = psum_pool.tile([128, n], F32, name="ps_fill", tag="ps")
                for _ in range(nf):
                    nc.tensor.matmul(
                        psf[:, 0:512], scratch[0:1, 0:128], scratch[0:1, :],
                        start=True, stop=True,
                    )

            FILL = {1: 10, 2: 10, 3: 10}
            for q in range(min(PIPE, NQ)):
                input_stage(q, split=(q <= 1))
            for q in range(NQ):
                if q + PIPE < NQ:
                    input_stage(q + PIPE, split=False)
                mm_stage(q)
                if q in FILL:
                    fillers(FILL[q])

    nc.compile()
    return nc


def host_prep(weight, bias):
    """Host-side static weight prep: s_w, ternary pre-transposed weight, bias.

    s_w must match the reference's fp32 mean reduction (jnp.mean on f32) to
    ~1 ulp, so it is computed with the same jax op on CPU when available.
    The ternarization then reproduces the reference exactly: fp32 multiply
    by s_w, fp32 round-half-even, clip to [-1, 1].
    """
    import ml_dtypes

    w = np.ascontiguousarray(weight, dtype=np.float32)
    try:
        import jax
        import jax.numpy as jnp

        with jax.default_device(jax.devices("cpu")[0]):
            mean_abs = np.float32(
                jax.device_get(jnp.mean(jnp.abs(jnp.asarray(w, dtype=jnp.float32))))
            )
    except Exception:
        mean_abs = np.float32(np.mean(np.abs(w), dtype=np.float32))
    mean_c = np.maximum(mean_abs, np.float32(EPS))
    sw = np.float32(1.0) / mean_c          # s_w, the weight quant scale
    k1 = (np.float32(1.0) / sw) / np.float32(127.0)  # output scale = cc * k1

    qw = np.clip(np.round(w * sw), -1.0, 1.0).astype(np.float32)  # [n, k] ternary
    # rhs layout [128 kpart, KT, NT, 128n]
    KT, NT = K // 128, N // 128
    qwt = (
        qw.T.reshape(KT, 128, NT, 128)       # [kt, kpart, nt, j]
        .transpose(1, 0, 2, 3)               # [kpart, kt, nt, j]
        .reshape(128, KT * N)
        .astype(ml_dtypes.bfloat16)
    )

    b = np.asarray(bias, dtype=np.float32).astype(ml_dtypes.bfloat16)
    biasb = np.tile(b[None, :], (128, 1))

    consts = np.zeros((128, 2), np.float32)
    consts[:, 0] = k1
    return qwt.copy(), biasb.copy(), consts


_NC_CACHE = {}


def _get_nc():
    if "nc" not in _NC_CACHE:
        _NC_CACHE["nc"] = build()
    return _NC_CACHE["nc"]


def make_in_maps(x, weight, bias):
    import ml_dtypes

    x = np.ascontiguousarray(np.asarray(x, dtype=np.float32)).astype(ml_dtypes.bfloat16)
    qwt, biasb, consts = host_prep(weight, bias)
    return [
        {"x": x[i], "qwt": qwt, "biasb": biasb, "consts": consts}
        for i in range(N_CORES)
    ]


def kernel(x, weight, bias, **kwargs):
    nc = _get_nc()
    in_maps = make_in_maps(x, weight, bias)
    last_err = None
    for _attempt in range(3):
        try:
            res = run_bass_kernel_spmd(nc, in_maps, list(range(N_CORES)))
            return np.stack(
                [res.results[i]["out"].astype(np.float32) for i in range(N_CORES)],
                axis=0,
            )
        except Exception as e:  # transient NRT device errors: retry
            last_err = e
    raise last_err


# revision 28
# speedup vs baseline: 1.0520x; 1.0520x over previous
"""BitLinear (fake-quant straight-through) Trainium2 kernel.

Math (per the reference nn module):
  dqx = round(x * s_x) / s_x         s_x = 127 / clip(rowabsmax(x), 1e-5)   (per token row)
  dqw = clip(round(w * s_w), -1, 1) / s_w    s_w = 1 / clip(mean(|w|), 1e-5)  (per tensor)
  out = dqx @ dqw.T + bias

Key facts this kernel exploits:
  * round(x*s_x) is an integer in [-127, 127] and clip(round(w*s_w)) is in
    {-1, 0, 1}; both are EXACT in bf16, and the matmul accumulates in fp32
    PSUM where all partial sums are exact integers.  The heavy matmul runs at
    bf16 PE rate; the per-token scale is applied to the exact integer matmul
    result at PSUM-evacuation time.
  * round-half-even == fp32 RNE, so `round(v)` is computed exactly as
    `(v + 1.5*2^23) - 1.5*2^23` with two fp32 ALU stages.
  * The tolerance gate is 2e-2; measured end-to-end error of this kernel is
    ~8.6e-3, dominated by two deliberate approximations: x is shipped to the
    device as bf16 (halves the input DMA; perturbs ~6% of the round-to-int
    decisions by +-1 step), and the output is stored as bf16 (+-2^-9
    relative) then upcast on the host.

Sharding: data parallel over the batch dim; core i computes batch element i
with the full weight.  No collectives; the host scatters x and gathers out.

Host-side static prep (weights are per-call constants): s_w (computed with
the same fp32 jnp.mean as the reference -- a 1e-6 error in s_w flips ternary
weights), the ternary weight pre-transposed into the matmul rhs layout
(bf16, exact), the bias broadcast row, and k1 = 1/(127*s_w).

Performance structure (~172-180 us/core; 512 matmuls x 216 ns = 110.6 us PE
floor, plus ~16 us fixed framework preamble/teardown):
  * Tokens are processed in quads (4 x 128 = 512 tokens, 8 quads/core); the
    coarse granularity keeps the DMA count at ~3 per 13.8 us PE slot, which
    keeps the tile framework's recycled DMA-completion semaphores far apart
    (fine-grained variants serialized on those recycle barriers).
  * The PE_HAM clock gate runs the PE at 1.2 GHz until it has been busy
    ~3.4 us, and re-throttles after any ~3.4 us idle gap -- so the matmul
    stream must never gap, and 40 dummy matmuls on scratch data pre-warm the
    gate through the input lead-in.
  * The input stage for quad q+3 is emitted alongside the matmuls for quad q;
    the first two quads are processed in per-128-token slices (with the
    -MAGIC convert on DVE) so the PE can start ~14 us in; steady-state quads
    use batched ops (convert on the scalar engine).
  * Engine queue steady-state budgets per 13.8 us quad slot: DVE ~9 us
    (absmax reduce, scales, round, 2 bias adds), ACT ~9.3 us (4 converts,
    4 wide PSUM evacuations), Pool ~8 us (2 bias adds, 4 output stores),
    sync ~8 us (x load + quad transpose dispatch).  PSUM uses all 8 banks
    (4 x [128,1024] tiles).
  * DMA rings: sync carries x loads + qx transposes, gpsimd carries the
    one-time weight load + output stores, scalar carries the lead-in x
    slices (so the DMA-semaphore ring never chains a transpose behind a
    steady-state x load).
"""

import numpy as np

from concourse import bacc, bass, mybir, tile
from concourse.bass_utils import run_bass_kernel_spmd

F32 = mybir.dt.float32
BF16 = mybir.dt.bfloat16
ALU = mybir.AluOpType
ACTF = mybir.ActivationFunctionType

MAGIC = 12582912.0  # 1.5 * 2**23: fp32 RNE round-to-integer constant
EPS = 1e-05

B, S, K, N = 8, 4096, 1024, 1024
N_CORES = 8
QS = 4       # token tiles per quad
PIPE = 3     # input-stage lookahead (quads)
WARMUP = 48  # dummy matmuls: pre-warm the PE HAM clock gate through
             # the lead-in (the first two quads are processed per-slice)


def build(s_tokens=S, k=K, n=N):
    """Build the single-core SPMD program: x[s_tokens,k] @ w[n,k]^T quantized."""
    nc = bacc.Bacc("TRN2", target_bir_lowering=False, debug=False)

    KT = k // 128          # contraction tiles
    NT = n // 128          # weight row tiles
    NH = n // 512          # psum-bank halves of the output feature dim
    NQ = s_tokens // (128 * QS)  # quads

    x_d = nc.dram_tensor("x", [s_tokens, k], BF16, kind="ExternalInput").ap()
    # qwt: host-ternarized weight, pre-transposed to the rhs layout
    # [kpart, kt, nt, n128]: element (p, kt, nt, j) = qw[n=nt*128+j, k=kt*128+p]
    qwt_d = nc.dram_tensor("qwt", [128, KT * n], BF16, kind="ExternalInput").ap()
    # bias broadcast to all 128 partitions (bf16)
    biasb_d = nc.dram_tensor("biasb", [128, n], BF16, kind="ExternalInput").ap()
    consts_d = nc.dram_tensor("consts", [128, 2], F32, kind="ExternalInput").ap()
    out_d = nc.dram_tensor("out", [s_tokens, n], BF16, kind="ExternalOutput").ap()

    x_q = x_d.rearrange("(q s p) k -> q p s k", s=QS, p=128)
    out_q = out_d.rearrange("(q s p) n -> q p s n", s=QS, p=128)

    with tile.TileContext(nc) as tc:
        with (
            tc.tile_pool(name="static", bufs=1) as static,
            tc.tile_pool(name="xpool", bufs=4) as xpool,
            tc.tile_pool(name="qpool", bufs=3) as qpool,
            tc.tile_pool(name="qtpool", bufs=4) as qtpool,
            tc.tile_pool(name="opool", bufs=4) as opool,
            tc.tile_pool(name="vpool", bufs=5) as vpool,
            tc.tile_pool(name="psum", bufs=4, space="PSUM") as psum_pool,
        ):
            # PE warmup scratch: memset by gpsimd, consumed by dummy matmuls
            scratch = static.tile([128, 512], BF16)
            nc.gpsimd.memset(scratch[:], 0.0)

            consts = static.tile([128, 2], F32)
            nc.scalar.dma_start(consts[:], consts_d[:])
            k1 = consts[:, 0:1]  # (1/s_w) / 127  (output scale factor)

            # weight rhs [128, kt, nt, 128] + bias on the gpsimd ring
            # (one-time; keeps the sync ring free for x loads + transposes)
            qwt = static.tile([128, KT, NT, 128], BF16)
            qwt_f = qwt[:].rearrange("p kt nt j -> p (kt nt j)")
            CH = KT * n // 2
            nc.gpsimd.dma_start(qwt_f[:, 0:CH], qwt_d[:, 0:CH])
            biasb = static.tile([128, n], BF16)
            nc.gpsimd.dma_start(biasb[:], biasb_d[:])
            nc.gpsimd.dma_start(qwt_f[:, CH:], qwt_d[:, CH:])

            # dummy matmuls: keep the PE busy through the input lead-in so
            # the HAM clock gate is warm (2.4 GHz) when the real stream
            # starts.  K=1 operands: PE-busy time is set by the 512 output
            # columns (216 ns each) but the moving operand is a single
            # partition row (~1 KB), so the dummies do not steal SBUF
            # bandwidth from the concurrent input-prep engines.
            ps_warm = psum_pool.tile([128, n], F32, name="ps_warm", tag="ps")
            for _ in range(WARMUP):
                nc.tensor.matmul(
                    ps_warm[:, 0:512], scratch[0:1, 0:128], scratch[0:1, :],
                    start=True, stop=True,
                )

            xs, fss, qxTs = {}, {}, {}

            def input_stage(q, split):
                """Load + quantize + transpose quad q.

                split=True processes the quad in 128-token slices (shorter
                dependency chains -- used for the lead-in quads); False uses
                one batched op per stage (fewer instructions, steady state).
                """
                x_s = xs[q] = xpool.tile([128, QS, k], BF16, name="x_s")
                cc = vpool.tile([128, QS], F32, name="cc")
                rc = vpool.tile([128, QS], F32, name="rc")
                ss = vpool.tile([128, QS], F32, name="ss")
                fs = fss[q] = vpool.tile([128, QS], F32, name="fs")
                xr = qpool.tile([128, QS, k], F32, name="xr")
                qx = qpool.tile([128, QS, k], BF16, name="qx")
                qxT = qxTs[q] = qtpool.tile([128, QS, KT, 128], BF16, name="qxT")

                ranges = [(s, s + 1) for s in range(QS)] if split else [(0, QS)]
                xeng = nc.scalar if split else nc.sync
                for lo, hi in ranges:
                    xeng.dma_start(x_s[:, lo:hi], x_q[q][:, lo:hi])
                    c = vpool.tile([128, hi - lo], F32, name=f"c{lo}" if split else "c")
                    nc.vector.tensor_reduce(
                        c[:], x_s[:, lo:hi], mybir.AxisListType.X, ALU.max,
                        apply_absolute_value=True,
                    )
                    nc.vector.tensor_scalar_max(cc[:, lo:hi], c[:], EPS)
                    nc.vector.reciprocal(rc[:, lo:hi], cc[:, lo:hi])
                    nc.vector.tensor_scalar_mul(ss[:, lo:hi], rc[:, lo:hi], 127.0)
                    nc.vector.tensor_scalar_mul(fs[:, lo:hi], cc[:, lo:hi], k1)
                    # round(x*s_x) via magic constant into an f32 scratch
                    # (per 128-token slice: the scalar operand is per-partition)
                    for s in range(lo, hi):
                        nc.vector.tensor_scalar(
                            xr[:, s, :], x_s[:, s, :], ss[:, s:s + 1], MAGIC,
                            ALU.mult, ALU.add,
                        )
                    for s in range(lo, hi):
                        if split:
                            nc.vector.tensor_scalar_sub(qx[:, s], xr[:, s], MAGIC)
                        else:
                            nc.scalar.activation(
                                qx[:, s], xr[:, s], ACTF.Copy, bias=-MAGIC,
                            )
                    # xbar transpose: [128s, k] -> [128k, KT, 128s] per slice
                    nc.sync.dma_start_transpose(qxT[:, lo:hi], qx[:, lo:hi])

            def mm_stage(q):
                qxT = qxTs.pop(q)
                fs = fss.pop(q)
                outs = opool.tile([128, QS, n], BF16, name="outs")
                for s in range(QS):
                    ps = psum_pool.tile([128, n], F32, name="ps", tag="ps")
                    for kt in range(KT):
                        for h in range(NH):
                            nc.tensor.matmul(
                                ps[:, h * 512:(h + 1) * 512],
                                qxT[:, s, kt, :],
                                qwt[:, kt, 4 * h:4 * h + 4, :],
                                start=(kt == 0),
                                stop=(kt == KT - 1),
                            )
                    nc.scalar.activation(
                        outs[:, s, :], ps[:], ACTF.Copy, scale=fs[:, s:s + 1],
                    )
                    beng = nc.gpsimd if s % 2 == 0 else nc.vector
                    beng.tensor_add(outs[:, s, :], outs[:, s, :], biasb[:])
                    nc.gpsimd.dma_start(out_q[q][:, s], outs[:, s])

            for q in range(min(PIPE, NQ)):
                input_stage(q, split=(q <= 1))
            for q in range(NQ):
                if q + PIPE < NQ:
                    input_stage(q + PIPE, split=False)
                mm_stage(q)

    nc.compile()
    return nc


def host_prep(weight, bias):
    """Host-side static weight prep: s_w, ternary pre-transposed weight, bias.

    s_w must match the reference's fp32 mean reduction (jnp.mean on f32) to
    ~1 ulp, so it is computed with the same jax op on CPU when available.
    The ternarization then reproduces the reference exactly: fp32 multiply
    by s_w, fp32 round-half-even, clip to [-1, 1].
    """
    import ml_dtypes

    w = np.ascontiguousarray(weight, dtype=np.float32)
    try:
        import jax
        import jax.numpy as jnp

        with jax.default_device(jax.devices("cpu")[0]):
            mean_abs = np.float32(
                jax.device_get(jnp.mean(jnp.abs(jnp.asarray(w, dtype=jnp.float32))))
            )
    except Exception:
        mean_abs = np.float32(np.mean(np.abs(w), dtype=np.float32))
    mean_c = np.maximum(mean_abs, np.float32(EPS))
    sw = np.float32(1.0) / mean_c          # s_w, the weight quant scale
    k1 = (np.float32(1.0) / sw) / np.float32(127.0)  # output scale = cc * k1

    qw = np.clip(np.round(w * sw), -1.0, 1.0).astype(np.float32)  # [n, k] ternary
    # rhs layout [128 kpart, KT, NT, 128n]
    KT, NT = K // 128, N // 128
    qwt = (
        qw.T.reshape(KT, 128, NT, 128)       # [kt, kpart, nt, j]
        .transpose(1, 0, 2, 3)               # [kpart, kt, nt, j]
        .reshape(128, KT * N)
        .astype(ml_dtypes.bfloat16)
    )

    b = np.asarray(bias, dtype=np.float32).astype(ml_dtypes.bfloat16)
    biasb = np.tile(b[None, :], (128, 1))

    consts = np.zeros((128, 2), np.float32)
    consts[:, 0] = k1
    return qwt.copy(), biasb.copy(), consts


_NC_CACHE = {}


def _get_nc():
    if "nc" not in _NC_CACHE:
        _NC_CACHE["nc"] = build()
    return _NC_CACHE["nc"]


def make_in_maps(x, weight, bias):
    import ml_dtypes

    x = np.ascontiguousarray(np.asarray(x, dtype=np.float32)).astype(ml_dtypes.bfloat16)
    qwt, biasb, consts = host_prep(weight, bias)
    return [
        {"x": x[i], "qwt": qwt, "biasb": biasb, "consts": consts}
        for i in range(N_CORES)
    ]


def kernel(x, weight, bias, **kwargs):
    nc = _get_nc()
    in_maps = make_in_maps(x, weight, bias)
    last_err = None
    for _attempt in range(3):
        try:
            res = run_bass_kernel_spmd(nc, in_maps, list(range(N_CORES)))
            return np.stack(
                [res.results[i]["out"].astype(np.float32) for i in range(N_CORES)],
                axis=0,
            )
        except Exception as e:  # transient NRT device errors: retry
            last_err = e
    raise last_err
